# revision 9
# baseline (speedup 1.0000x reference)
"""Trainium2 Bass kernel for nn_CombinedLoss (dice + cross-entropy over [8,23,512,512] logits).

Sharding: pure data-parallel — one batch image per NeuronCore (8 cores).
Each core computes per-class partial sums (sum_p, intersection) and CE partial
sums (sum logZ, sum x_t); the host reduces partials across cores and applies
the final scalar formula (the "all-reduce + combine" step).

Self-contained: hardcodes shapes; only needs the in-container concourse repo.
"""

import sys

for _p in ("/opt/trn_rl_repo", "/root/.axon_site/_ro/trn_rl_repo"):
    if _p not in sys.path:
        sys.path.insert(0, _p)

import numpy as np
import ml_dtypes

import concourse.bass as bass  # noqa: F401
import concourse.bacc as bacc
import concourse.tile as tile
import concourse.mybir as mybir
from concourse.bass_utils import run_bass_kernel_spmd
from concourse.dve_ops import RECIPROCAL_APPROX_FAST, RECIP_APPROX_FAST_CONSTS

F32 = mybir.dt.float32
BF16 = mybir.dt.bfloat16

C = 23          # classes
G = 4           # class-groups per compute tile (4*23 = 92 partitions)
GC = G * C      # 92
AUG = GC + G    # 96: 92 exp rows + 4 target rows riding the same transpose
F = 512         # free columns per class-major tile (pixels per group)
CH = 128        # pixels per transpose chunk
J = F // CH     # chunks per tile = 4
S16 = J * G     # pixel-major slots per tile = 16
SC = S16 * C    # 368
SMOOTH = 1e-05
WEIGHT = 0.5

N_CORES = 8
B, H, W = 8, 512, 512
HW_FULL = H * W  # pixels per core (one image per core)


def build_loss_kernel(nc, tc, ctx, HW):
    """Emit the per-core kernel into TileContext tc. HW must be T*G*F."""
    T = HW // (G * F)            # class-major tiles
    SUPER = min(32, T)           # tiles per batched-reciprocal phase
    NS = T // SUPER              # supers
    assert T * G * F == HW and NS * SUPER == T

    x = nc.dram_tensor("x", [C, HW], F32, kind="ExternalInput").ap()
    tb_in = nc.dram_tensor("tb", [HW], BF16, kind="ExternalInput").ap()
    # [2, 16, 368]: [0] = sum_p matmul accumulator, [1] = intersection
    pi_out = nc.dram_tensor("pi", [2, S16, SC], F32, kind="ExternalOutput").ap()
    # [2, 128, NS]: [0] = per-super sum(log Z), [1] = per-super sum(log E_t)
    ln_out = nc.dram_tensor("ln", [2, 128, NS], F32, kind="ExternalOutput").ap()

    const_pool = ctx.enter_context(tc.tile_pool(name="const", bufs=1))
    xpool = ctx.enter_context(tc.tile_pool(name="x", bufs=3))
    epool = ctx.enter_context(tc.tile_pool(name="e", bufs=3))
    tpsum = ctx.enter_context(tc.tile_pool(name="tpsum", bufs=4, space="PSUM"))
    etpool = ctx.enter_context(tc.tile_pool(name="et", bufs=SUPER + 4))
    meppool = ctx.enter_context(tc.tile_pool(name="mep", bufs=SUPER + 4))
    ohtpool = ctx.enter_context(tc.tile_pool(name="oht", bufs=3))
    accpool = ctx.enter_context(tc.tile_pool(name="acc", bufs=1))
    accpsum = ctx.enter_context(tc.tile_pool(name="accpsum", bufs=1, space="PSUM"))
    scratch = ctx.enter_context(tc.tile_pool(name="scr", bufs=2))

    eq = mybir.AluOpType.is_equal
    add = mybir.AluOpType.add
    mult = mybir.AluOpType.mult

    # ---- static constants ----
    ident_i = const_pool.tile([128, 128], mybir.dt.int32)
    nc.gpsimd.iota(ident_i[:], pattern=[[1, 128]], base=0, channel_multiplier=-1)
    ident = const_pool.tile([128, 128], BF16)
    nc.vector.tensor_scalar(ident[:], ident_i[:], 0, None, eq)
    # iota over classes, tiled [128, S16*C]: value = c at free index (s*C + c)
    iota_i = const_pool.tile([128, SC], mybir.dt.int32)
    nc.gpsimd.iota(iota_i[:], pattern=[[0, S16], [1, C]], base=0, channel_multiplier=0)
    iota_b = const_pool.tile([128, SC], BF16)
    nc.vector.tensor_copy(iota_b[:], iota_i[:])

    # ---- whole-core accumulators ----
    ZtB = accpool.tile([128, T * S16], F32)    # per-pixel Z (softmax denom)
    EtB = accpool.tile([128, T * S16], F32)    # per-pixel exp(x_t)
    rTb = accpool.tile([128, T * S16], BF16)   # per-pixel 1/Z (bf16 for matmul)
    lnZa = accpool.tile([128, NS], F32)
    lnEa = accpool.tile([128, NS], F32)

    psumP = accpsum.tile([S16, SC], F32)
    psumI = accpsum.tile([S16, SC], F32)

    for s in range(NS):
        ets, meps = [], []
        for it in range(SUPER):
            i = s * SUPER + it
            # ---- load class-major tile [92, 512] ----
            tx = xpool.tile([GC, F], F32, name=f"tx{i}", tag="tx")
            xv = x[:, i * (G * F):(i + 1) * (G * F)].rearrange(
                "c (g f) -> g c f", g=G, f=F)
            nc.sync.dma_start(out=tx[:], in_=xv)
            # ---- exp rows 0..91; target rows 92..95 via DMA ----
            te = epool.tile([AUG, F], BF16, name=f"te{i}", tag="te")
            nc.scalar.activation(te[:GC, :], tx[:],
                                 mybir.ActivationFunctionType.Exp)
            tbv = tb_in[i * (G * F):(i + 1) * (G * F)].rearrange(
                "(g f) -> g f", g=G, f=F)
            nc.sync.dma_start(out=te[GC:AUG, :], in_=tbv)
            # ---- transpose chunks to pixel-major [128, 4*96] (bf16 psum) ----
            pt = tpsum.tile([128, J * AUG], BF16, name=f"pt{i}", tag="pt")
            for j in range(J):
                nc.tensor.transpose(
                    pt[:, j * AUG:(j + 1) * AUG],
                    te[:, j * CH:(j + 1) * CH],
                    ident[:AUG, :AUG],
                )
            eta = etpool.tile([128, J * AUG], BF16, name=f"et{i}", tag="et")
            nc.vector.tensor_copy(eta[:], pt[:])
            # views: E part [128, (j), (g c)], t part [128, (j), (g)]
            ev = eta[:].rearrange("p (j q) -> p j q", j=J)[:, :, 0:GC]
            tv = eta[:].rearrange("p (j q) -> p j q", j=J)[:, :, GC:AUG]
            # ---- one-hot of t, masked exp, reductions ----
            tT = scratch.tile([128, S16], BF16, name=f"tT{i}", tag="tT", bufs=3)
            nc.vector.tensor_copy(tT[:], tv)
            oht = ohtpool.tile([128, SC], BF16, name=f"oht{i}", tag="oht")
            tbc = tT[:].unsqueeze(2).broadcast_to([128, S16, C])
            nc.vector.tensor_tensor(
                oht[:].rearrange("p (s c) -> p s c", c=C), tbc,
                iota_b[:].rearrange("p (s c) -> p s c", c=C), eq)
            mep = meppool.tile([128, SC], BF16, name=f"mep{i}", tag="mep")
            ev4 = ev.rearrange("p j (g c) -> p j g c", c=C)
            nc.vector.tensor_tensor(
                mep[:].rearrange("p (j g c) -> p j g c", g=G, c=C),
                oht[:].rearrange("p (j g c) -> p j g c", g=G, c=C), ev4, mult)
            nc.vector.tensor_reduce(
                ZtB[:, i * S16:(i + 1) * S16], ev4,
                axis=mybir.AxisListType.X, op=add)
            nc.vector.tensor_reduce(
                EtB[:, i * S16:(i + 1) * S16],
                mep[:].rearrange("p (s c) -> p s c", c=C),
                axis=mybir.AxisListType.X, op=add)
            ets.append(eta)
            meps.append(mep)

        # ---- batched per-super: reciprocal, bf16 cast, CE log-sums ----
        sl = slice(s * SUPER * S16, (s + 1) * SUPER * S16)
        rT32 = scratch.tile([128, SUPER * S16], F32, name=f"r32_{s}", tag="r32")
        nc.vector._custom_dve(
            RECIPROCAL_APPROX_FAST, out=rT32[:], in0=ZtB[:, sl],
            **RECIP_APPROX_FAST_CONSTS)
        nc.vector.tensor_copy(rTb[:, sl], rT32[:])
        lnscr = scratch.tile([128, SUPER * S16], BF16, name=f"lnscr{s}",
                             tag="lnscr")
        nc.scalar.activation(
            lnscr[:], ZtB[:, sl], mybir.ActivationFunctionType.Ln,
            accum_out=lnZa[:, s:s + 1])
        nc.scalar.activation(
            lnscr[:], EtB[:, sl], mybir.ActivationFunctionType.Ln,
            accum_out=lnEa[:, s:s + 1])

        # ---- per-tile matmuls: accumulate sum_p and intersection ----
        for it in range(SUPER):
            i = s * SUPER + it
            lhs = rTb[:, i * S16:(i + 1) * S16]
            ev = ets[it][:].rearrange("p (j q) -> p j q", j=J)[:, :, 0:GC]
            nc.tensor.matmul(psumP[:].rearrange("m (j gc) -> m j gc", j=J),
                             lhsT=lhs, rhs=ev,
                             start=(i == 0), stop=(i == T - 1))
            nc.tensor.matmul(psumI[:], lhsT=lhs, rhs=meps[it][:],
                             start=(i == 0), stop=(i == T - 1))

    # ---- write outputs ----
    outPI = accpool.tile([S16, 2, SC], F32)
    nc.vector.tensor_copy(outPI[:, 0, :], psumP[:])
    nc.vector.tensor_copy(outPI[:, 1, :], psumI[:])
    nc.sync.dma_start(out=pi_out.rearrange("a m n -> m a n"), in_=outPI[:])
    nc.sync.dma_start(out=ln_out[0], in_=lnZa[:])
    nc.sync.dma_start(out=ln_out[1], in_=lnEa[:])


_KERNEL_CACHE = {}


def _get_compiled(HW):
    if HW in _KERNEL_CACHE:
        return _KERNEL_CACHE[HW]
    from contextlib import ExitStack
    nc = bacc.Bacc("TRN2", target_bir_lowering=False, debug=False,
                   num_devices=N_CORES)
    with tile.TileContext(nc) as tc:
        with ExitStack() as ctx:
            build_loss_kernel(nc, tc, ctx, HW)
    nc.compile()
    _KERNEL_CACHE[HW] = nc
    return nc


def combine_partials(results, targets, HW, n_px_total):
    """Host-side all-reduce of per-core partials + final scalar combine.

    Note psumP rows were laid out [m, (j, g*c)] with rhs [128, j, 92]; the
    matmul flattens rhs free dims to (j, g, c) order, matching slot m = j*4+g
    blocks at column m*23..m*23+23 exactly like psumI.
    """
    sumP = np.zeros(C, np.float64)
    inter = np.zeros(C, np.float64)
    ce_sum = 0.0
    for res in results:
        pi = res["pi"].astype(np.float64)      # [2, 16, 368]
        ln = res["ln"].astype(np.float64)      # [2, 128, NS]
        for m in range(S16):
            sumP += pi[0, m, m * C:(m + 1) * C]
            inter += pi[1, m, m * C:(m + 1) * C]
        ce_sum += ln[0].sum() - ln[1].sum()    # sum(logZ) - sum(x_t)
    counts = np.bincount(targets.reshape(-1).astype(np.int64), minlength=C
                         ).astype(np.float64)
    dice = (2.0 * inter + SMOOTH) / (sumP + counts + SMOOTH)
    dice_loss = (1.0 - dice).mean()
    ce = ce_sum / n_px_total
    return np.float32(WEIGHT * dice_loss + (1.0 - WEIGHT) * ce)


def kernel(predictions: np.ndarray, targets: np.ndarray) -> np.ndarray:
    assert predictions.shape == (B, C, H, W)
    HW = HW_FULL
    nc = _get_compiled(HW)
    in_maps = []
    for b in range(N_CORES):
        xb = np.ascontiguousarray(predictions[b].reshape(C, HW), dtype=np.float32)
        tb = np.ascontiguousarray(
            targets[b, 0].reshape(HW).astype(ml_dtypes.bfloat16))
        in_maps.append({"x": xb, "tb": tb})
    res = run_bass_kernel_spmd(nc, in_maps, core_ids=list(range(N_CORES)))
    loss = combine_partials(res.results, targets, HW, B * HW)
    return np.array(loss, dtype=np.float32)


if __name__ == "__main__":
    rng = np.random.default_rng(0)
    preds = rng.standard_normal((B, C, H, W), dtype=np.float32)
    tgts = rng.integers(0, C, (B, 1, H, W)).astype(np.int64)
    print(kernel(predictions=preds, targets=tgts))
